# revision 15
# baseline (speedup 1.0000x reference)
"""NT-Xent (SimCLR) loss on 8 trn2 NeuronCores.

Math (matches the jax reference):
    z  = concat(z_i, z_j)                  [2B, D], 2B=8192, D=256
    zn = z / ||z||                         (row-normalize)
    sim = zn @ zn.T                        [2B, 2B]
    logits = where(diag, -9e15, sim) / T
    loss = -mean(log_softmax(logits)[r, pair(r)])

Sharding: rows are split across 8 cores (1024 rows each).  Each core
receives the FULL z with its own rows permuted to the front ("z_perm"),
plus the positive-pair rows for its block ("z_pairs").  The permutation
makes the kernel perfectly SPMD: the diagonal (self-similarity) always
falls in columns [128*t, 128*t+128) of row-tile t, so one NEFF serves
all 8 cores and no collectives are needed.  Because logits are bounded
by 1/T (cosine in [-1,1]), log-softmax uses the FIXED shift m = 1/T —
no max pass is needed:
    lse_r   = 1/T + log( sum_c exp(sim_rc/T - 1/T) )   (diag masked out)
    loss_r  = lse_r - d_r/T,   d_r = zn_r . zn_pair(r)
Each core returns its [128, 8] tile of (log s_r - d_r/T); the host adds
1/T and takes the mean.
"""

import numpy as np

B = 4096
D = 256
N = 2 * B            # 8192 rows total
P = 128              # SBUF partitions
NCORES = 8
RPC = N // NCORES    # 1024 rows per core
MT = RPC // P        # 8 row-tiles per core
NT = N // P          # 64 row-tiles total
TEMP = 0.07
SCALE = 1.0 / TEMP
NEG = -1.0e6         # additive diagonal mask (pre-temperature)

# Gram matmul precision mode: "f32r" (full-rate fp32r) or "f32" (exact,
# 4x slower).
MM_MODE = "f32r"

# Debug: stop after phase N (1=norms, 2=normalize+transpose, 3=gram+exp,
# 4=everything).  Early phases copy a probe value into the output.
STOP_AFTER = 4

_CACHED_NC = None


def _build_nc():
    import concourse.bacc as bacc
    import concourse.mybir as mybir
    import concourse.tile as tile
    from concourse.masks import make_identity

    f32 = mybir.dt.float32
    f32r = mybir.dt.float32r
    AF = mybir.ActivationFunctionType
    ALU = mybir.AluOpType

    nc = bacc.Bacc(name="ntxent")
    zp = nc.dram_tensor("z_perm", [N, D], f32, kind="ExternalInput")
    zq = nc.dram_tensor("z_pairs", [RPC, D], f32, kind="ExternalInput")
    out = nc.dram_tensor("row_loss", [P, MT], f32, kind="ExternalOutput")

    with tile.TileContext(nc) as tc:
        with (
            tc.tile_pool(name="big", bufs=1) as big,
            tc.tile_pool(name="small", bufs=1) as small,
            tc.tile_pool(name="scr", bufs=4) as scr,
            tc.tile_pool(name="expool", bufs=3) as expool,
            tc.tile_pool(name="psp", bufs=2, space="PSUM") as psp,
        ):
            zrows = big.tile([P, NT, D], f32)    # row-major z, normalized in place
            zpair = big.tile([P, MT, D], f32)
            znt0 = big.tile([P, N], f32)         # zn.T rows 0:128 of D
            znt1 = big.tile([P, N], f32)         # zn.T rows 128:256 of D
            znt = [znt0, znt1]

            SSa = small.tile([P, NT], f32)
            SSp = small.tile([P, MT], f32)
            RNa = small.tile([P, NT], f32)
            RNp = small.tile([P, MT], f32)
            SUMS = small.tile([P, MT * 4], f32)
            Ssum = small.tile([P, MT], f32)
            Dd = small.tile([P, MT], f32)
            LOGS = small.tile([P, MT], f32)
            LOSS = small.tile([P, MT], f32)
            ident = small.tile([P, P], f32)
            dmask = small.tile([P, P], f32)
            nbias = small.tile([P, 1], f32)
            c15 = small.tile([P, NT], f32)   # 1.5 constant (Newton step)

            nc.vector.memset(nbias[:], -SCALE)
            nc.vector.memset(c15[:], 1.5)
            make_identity(nc, ident[:])
            nc.gpsimd.memset(dmask[:], 0.0)
            nc.gpsimd.affine_select(
                out=dmask[:],
                in_=dmask[:],
                compare_op=ALU.not_equal,
                fill=NEG,
                base=0,
                pattern=[[-1, P]],
                channel_multiplier=1,
            )

            # ---- loads ----
            zp_r = zp[:].rearrange("(n p) d -> p n d", p=P)
            zq_r = zq[:].rearrange("(n p) d -> p n d", p=P)
            CH = 8
            for i0 in range(0, NT, CH):
                nc.sync.dma_start(
                    out=zrows[:, i0 : i0 + CH, :], in_=zp_r[:, i0 : i0 + CH, :]
                )
            nc.sync.dma_start(out=zpair[:], in_=zq_r[:])

            # ---- phase 1: row sum-of-squares (split across ACT and DVE) ----
            def sumsq(src, i, ss_col):
                s = scr.tile([P, D], f32, tag="sq")
                if i % 2 == 0:
                    nc.scalar.activation(
                        out=s[:], in_=src, func=AF.Square, accum_out=ss_col
                    )
                else:
                    nc.vector.tensor_mul(out=s[:], in0=src, in1=src)
                    nc.vector.reduce_sum(
                        out=ss_col, in_=s[:], axis=mybir.AxisListType.X
                    )

            for i in range(NT):
                sumsq(zrows[:, i, :], i, SSa[:, i : i + 1])
            for i in range(MT):
                sumsq(zpair[:, i, :], i, SSp[:, i : i + 1])

            # rn = 1/sqrt(ss), one Newton step to clean up ACT sqrt error
            def rsqrt_newton(SS_t, RN_t, w):
                sr = scr.tile([P, w], f32, tag=f"rs{w}")
                nc.scalar.activation(out=sr[:], in_=SS_t[:], func=AF.Sqrt)
                r0 = scr.tile([P, w], f32, tag=f"rs{w}b")
                nc.vector.reciprocal(out=r0[:], in_=sr[:])
                t1 = scr.tile([P, w], f32, tag=f"rs{w}c")
                nc.vector.tensor_mul(out=t1[:], in0=r0[:], in1=r0[:])
                nc.vector.tensor_mul(out=t1[:], in0=t1[:], in1=SS_t[:])
                # t1 = 1.5 - 0.5 * t1   (2-scalar tensor_scalar crashes HW;
                # use scalar_tensor_tensor with a 1.5-filled tile instead)
                nc.vector.scalar_tensor_tensor(
                    out=t1[:], in0=t1[:], scalar=-0.5, in1=c15[:, :w],
                    op0=ALU.mult, op1=ALU.add,
                )
                nc.vector.tensor_mul(out=RN_t[:], in0=r0[:], in1=t1[:])

            rsqrt_newton(SSa, RNa, NT)
            rsqrt_newton(SSp, RNp, MT)

            if STOP_AFTER <= 1:
                nc.vector.tensor_copy(out=LOSS[:], in_=RNa[:, :MT])

            # ---- phase 2: normalize in place, pair dots, transpose ----
            if STOP_AFTER >= 2:
                for i in range(NT):
                    nc.vector.tensor_scalar_mul(
                        out=zrows[:, i, :],
                        in0=zrows[:, i, :],
                        scalar1=RNa[:, i : i + 1],
                    )
                for i in range(MT):
                    nc.vector.tensor_scalar_mul(
                        out=zpair[:, i, :],
                        in0=zpair[:, i, :],
                        scalar1=RNp[:, i : i + 1],
                    )
                for i in range(MT):
                    s = scr.tile([P, D], f32, tag="sq")
                    nc.vector.tensor_mul(
                        out=s[:], in0=zrows[:, i, :], in1=zpair[:, i, :]
                    )
                    nc.vector.reduce_sum(
                        out=Dd[:, i : i + 1], in_=s[:], axis=mybir.AxisListType.X
                    )

                # zn.T via TensorE transposes, 4 tiles per PSUM bank, then evac
                for k in range(2):
                    for j in range(NT // 4):
                        pt = psp.tile([P, 512], f32, tag="mm")
                        for q in range(4):
                            i = 4 * j + q
                            nc.tensor.transpose(
                                out=pt[:, q * 128 : (q + 1) * 128],
                                in_=zrows[:, i, k * 128 : (k + 1) * 128],
                                identity=ident[:],
                            )
                        dst = znt[k][:, j * 512 : (j + 1) * 512]
                        if MM_MODE == "f32r":
                            dst = dst.bitcast(f32r)
                        if j % 2 == 0:
                            nc.scalar.copy(out=dst, in_=pt[:])
                        else:
                            nc.vector.tensor_copy(out=dst, in_=pt[:])

            if STOP_AFTER == 2:
                nc.vector.tensor_copy(out=LOSS[:], in_=znt0[:, :MT])

            # ---- phase 3: Gram row-block x all columns, fused exp-sum ----
            def mm_ap(t_, a, b):
                if MM_MODE == "f32r":
                    return t_[:, a:b].bitcast(f32r)
                return t_[:, a:b]

            NGRP = N // 2048  # 4 col groups of 2048
            if STOP_AFTER >= 3:
                for t in range(MT):
                    lhs = [mm_ap(znt[k], t * 128, (t + 1) * 128) for k in range(2)]
                    for g in range(NGRP):
                        ps = psp.tile([P, 2048], f32, tag="mm")
                        for q in range(4):
                            c0 = g * 2048 + q * 512
                            pq = ps[:, q * 512 : (q + 1) * 512]
                            nc.tensor.matmul(
                                pq, lhs[0], mm_ap(znt[0], c0, c0 + 512),
                                start=True, stop=False,
                            )
                            nc.tensor.matmul(
                                pq, lhs[1], mm_ap(znt[1], c0, c0 + 512),
                                start=False, stop=True,
                            )
                        if g == 0:
                            off = t * 128
                            nc.vector.tensor_add(
                                out=ps[:, off : off + 128],
                                in0=ps[:, off : off + 128],
                                in1=dmask[:],
                            )
                        es = expool.tile([P, 2048], f32, tag="es")
                        nc.scalar.activation(
                            out=es[:],
                            in_=ps[:],
                            func=AF.Exp,
                            bias=nbias[:],
                            scale=SCALE,
                            accum_out=SUMS[:, t * NGRP + g : t * NGRP + g + 1],
                        )

                sums_v = SUMS[:].rearrange("p (t g) -> p t g", g=NGRP)
                nc.vector.reduce_sum(
                    out=Ssum[:], in_=sums_v, axis=mybir.AxisListType.X
                )

            if STOP_AFTER == 3:
                nc.vector.tensor_copy(out=LOSS[:], in_=Ssum[:])

            # ---- phase 4: finalize log s - d/T ----
            if STOP_AFTER >= 4:
                nc.scalar.activation(out=LOGS[:], in_=Ssum[:], func=AF.Ln)
                nc.vector.scalar_tensor_tensor(
                    out=LOSS[:],
                    in0=Dd[:],
                    scalar=-SCALE,
                    in1=LOGS[:],
                    op0=ALU.mult,
                    op1=ALU.add,
                )

            nc.sync.dma_start(out=out[:], in_=LOSS[:])

    nc.finalize()
    return nc


def _get_nc():
    global _CACHED_NC
    if _CACHED_NC is None:
        _CACHED_NC = _build_nc()
    return _CACHED_NC


def make_in_maps(z_i, z_j):
    z = np.concatenate(
        [np.asarray(z_i, dtype=np.float32), np.asarray(z_j, dtype=np.float32)], axis=0
    )
    in_maps = []
    for c in range(NCORES):
        s0, s1 = c * RPC, (c + 1) * RPC
        z_perm = np.ascontiguousarray(
            np.concatenate([z[s0:s1], z[:s0], z[s1:]], axis=0)
        )
        p0 = (s0 + B) % N
        z_pairs = np.ascontiguousarray(z[p0 : p0 + RPC])
        in_maps.append({"z_perm": z_perm, "z_pairs": z_pairs})
    return in_maps


def finish(results):
    total = 0.0
    for r in results:
        total += float(np.sum(r["row_loss"].astype(np.float64)))
    return np.asarray(SCALE + total / N, dtype=np.float32)


def run_spmd(z_i, z_j, **kw):
    from concourse.bass_utils import run_bass_kernel_spmd

    in_maps = make_in_maps(z_i, z_j)
    return run_bass_kernel_spmd(_get_nc(), in_maps, core_ids=list(range(NCORES)), **kw)


def kernel(z_i, z_j):
    res = run_spmd(z_i, z_j)
    return finish(res.results)


if __name__ == "__main__":
    rng = np.random.default_rng(0)
    zi = rng.standard_normal((B, D), dtype=np.float32)
    zj = rng.standard_normal((B, D), dtype=np.float32)
    print(kernel(zi, zj))
